# revision 6
# baseline (speedup 1.0000x reference)
import numpy as np
import ml_dtypes

B, T = 4, 2048
TOK = 1024
C = 1024
H = 4096
E = 8
CAP = 384
N_CORES = 8
NT = TOK // 128
KC = C // 128
KH = H // 128
NCT = CAP // 128

_NC_CACHE = {}


def _build_nc():
    import concourse.bacc as bacc
    import concourse.mybir as mybir
    import concourse.tile as tile
    from concourse import bass
    from concourse.masks import make_identity

    dt = mybir.dt
    AF = mybir.ActivationFunctionType
    ALU = mybir.AluOpType
    F32, BF16, I16, I32, U32 = dt.float32, dt.bfloat16, dt.int16, dt.int32, dt.uint32

    nc = bacc.Bacc("TRN2", target_bir_lowering=False, debug=False,
                   num_devices=N_CORES)

    xr = nc.dram_tensor("xr", [TOK, C], BF16, kind="ExternalInput").ap()
    xtf = nc.dram_tensor("xtf", [C, TOK], F32, kind="ExternalInput").ap()
    xt = nc.dram_tensor("xt", [C, TOK], BF16, kind="ExternalInput").ap()
    wr = nc.dram_tensor("wr", [C, E], F32, kind="ExternalInput").ap()
    wfc = nc.dram_tensor("wfc", [E, C, H], BF16, kind="ExternalInput").ap()
    wpj = nc.dram_tensor("wpj", [E, H, C], BF16, kind="ExternalInput").ap()
    wfcs = nc.dram_tensor("wfcs", [C, H], BF16, kind="ExternalInput").ap()
    wpjs = nc.dram_tensor("wpjs", [H, C], BF16, kind="ExternalInput").ap()
    bfc = nc.dram_tensor("bfc", [E, H], F32, kind="ExternalInput").ap()
    bfcs = nc.dram_tensor("bfcs", [H], F32, kind="ExternalInput").ap()
    b9 = nc.dram_tensor("b9", [E + 1, C], F32, kind="ExternalInput").ap()
    y = nc.dram_tensor("y", [TOK, C], F32, kind="ExternalOutput").ap()
    tbl = nc.dram_tensor("tbl", [E * CAP, 2], F32, kind="Internal")
    tbl_ap = tbl.ap()

    with tile.TileContext(nc) as tc:
        with tc.tile_pool(name="pp", bufs=1) as pp, \
             tc.tile_pool(name="rt", bufs=1) as rt, \
             tc.tile_pool(name="rw", bufs=2) as rw, \
             tc.tile_pool(name="sh", bufs=2) as sh, \
             tc.tile_pool(name="te", bufs=2) as te, \
             tc.tile_pool(name="he", bufs=1) as he, \
             tc.tile_pool(name="wf", bufs=3) as wf, \
             tc.tile_pool(name="wp", bufs=2) as wp, \
             tc.tile_pool(name="os", bufs=1) as osp, \
             tc.tile_pool(name="psA", bufs=3, space="PSUM") as psA, \
             tc.tile_pool(name="psT", bufs=1, space="PSUM") as psT, \
             tc.tile_pool(name="psY", bufs=4, space="PSUM") as psY:

            xt_sb = pp.tile([128, KC, TOK], BF16)
            gidx = pp.tile([128, E * CAP // 16], I16)
            gval = pp.tile([128, E * NCT], F32)
            gateT = pp.tile([E + 1, TOK], F32)
            bias9 = pp.tile([E + 1, C], F32)
            bfc_sb = pp.tile([128, E * KH], F32)
            bfcs_sb = pp.tile([128, KH], F32)
            idn = pp.tile([128, 128], F32)
            ones_m = pp.tile([128, 128], F32)
            sut_m = pp.tile([128, 128], F32)
            iota8 = pp.tile([128, 8], F32)

            nc.sync.dma_start(out=xt_sb[:],
                              in_=xt.rearrange("(k p) t -> p k t", p=128))
            nc.sync.dma_start(out=bias9[:], in_=b9)
            nc.sync.dma_start(out=bfc_sb[:],
                              in_=bfc.rearrange("e (m p) -> p (e m)", p=128))
            nc.sync.dma_start(out=bfcs_sb[:],
                              in_=bfcs.rearrange("(m p) -> p m", p=128))
            make_identity(nc, idn[:])
            nc.gpsimd.memset(ones_m[:], 1.0)
            nc.gpsimd.memset(sut_m[:], 1.0)
            nc.gpsimd.affine_select(out=sut_m[:], in_=sut_m[:],
                                    compare_op=ALU.is_gt, fill=0.0, base=0,
                                    pattern=[[1, 128]], channel_multiplier=-1)
            iota8_i = rt.tile([128, 8], I32, tag="iotai")
            nc.gpsimd.iota(iota8_i[:], pattern=[[1, 8]], base=0,
                           channel_multiplier=0)
            nc.vector.tensor_copy(iota8[:], iota8_i[:])

            wr_sb = rt.tile([128, KC, E], F32, tag="wr")
            nc.sync.dma_start(out=wr_sb[:],
                              in_=wr.rearrange("(k p) e -> p k e", p=128))
            logit = rt.tile([128, NT, E], F32, tag="logit")
            for t in range(NT):
                xtf_t = rw.tile([128, KC, 128], F32, tag="xtf")
                nc.sync.dma_start(
                    out=xtf_t[:],
                    in_=xtf[:, t * 128:(t + 1) * 128]
                        .rearrange("(k p) t -> p k t", p=128))
                ps = psA.tile([128, E], F32, tag="acc")
                for k in range(KC):
                    nc.tensor.matmul(ps[:], lhsT=xtf_t[:, k, :],
                                     rhs=wr_sb[:, k, :],
                                     start=(k == 0), stop=(k == KC - 1))
                nc.vector.tensor_copy(logit[:, t, :], ps[:])

            mask = rt.tile([128, NT, E], F32, tag="mask")
            gden = rt.tile([128, NT, E + 1], F32, tag="gden")
            eq1a = rt.tile([128, NT, E], F32, tag="eq1")
            eq2a = rt.tile([128, NT, E], F32, tag="eq2")
            e1f = rt.tile([128, NT], F32, tag="e1f")
            e2f = rt.tile([128, NT], F32, tag="e2f")
            g1a = rt.tile([128, NT], F32, tag="g1")
            g2a = rt.tile([128, NT], F32, tag="g2")
            pos = rt.tile([128, NT, E], F32, tag="pos")
            for t in range(NT):
                m8 = rt.tile([128, 8], F32, tag="m8")
                i8 = rt.tile([128, 8], U32, tag="i8")
                nc.vector.max_with_indices(m8[:], i8[:], logit[:, t, :])
                nc.vector.tensor_copy(e1f[:, t:t + 1], i8[:, 0:1])
                nc.vector.tensor_copy(e2f[:, t:t + 1], i8[:, 1:2])
                d12 = rt.tile([128, 1], F32, tag="d12")
                nc.vector.tensor_tensor(d12[:], m8[:, 0:1], m8[:, 1:2],
                                        op=ALU.subtract)
                nc.scalar.activation(g1a[:, t:t + 1], d12[:], AF.Sigmoid)
                nc.scalar.activation(g2a[:, t:t + 1], d12[:], AF.Sigmoid,
                                     scale=-1.0)
                nc.vector.tensor_scalar(eq1a[:, t, :], iota8[:],
                                        e1f[:, t:t + 1], None,
                                        op0=ALU.is_equal)
                nc.vector.tensor_scalar(eq2a[:, t, :], iota8[:],
                                        e2f[:, t:t + 1], None,
                                        op0=ALU.is_equal)
                nc.vector.tensor_tensor(mask[:, t, :], eq1a[:, t, :],
                                        eq2a[:, t, :], op=ALU.add)
                tg1 = rt.tile([128, E], F32, tag="tg1")
                tg2 = rt.tile([128, E], F32, tag="tg2")
                nc.vector.tensor_scalar(tg1[:], eq1a[:, t, :],
                                        g1a[:, t:t + 1], None, op0=ALU.mult)
                nc.vector.tensor_scalar(tg2[:], eq2a[:, t, :],
                                        g2a[:, t:t + 1], None, op0=ALU.mult)
                nc.vector.tensor_tensor(gden[:, t, :E], tg1[:], tg2[:],
                                        op=ALU.add)
                nc.vector.memset(gden[:, t, E:E + 1], 1.0)

            for t in range(NT):
                ps = psA.tile([128, E], F32, tag="acc")
                for k in range(t + 1):
                    nc.tensor.matmul(ps[:],
                                     lhsT=(sut_m[:] if k == t else ones_m[:]),
                                     rhs=mask[:, k, :],
                                     start=(k == 0), stop=(k == t))
                nc.vector.tensor_copy(pos[:, t, :], ps[:])

            for t in range(NT):
                trp = psT.tile([E + 1, 128], F32, tag="tr")
                nc.tensor.transpose(trp[:], gden[:, t, :], idn[:])
                nc.vector.tensor_copy(gateT[0:E + 1, t * 128:(t + 1) * 128],
                                      trp[:])

            ztbl = rt.tile([128, E * CAP // 128, 2], F32, tag="ztbl")
            nc.gpsimd.memset(ztbl[:], 0.0)
            nc.sync.dma_start(
                out=tbl_ap.rearrange("(a p) c -> p a c", p=128),
                in_=ztbl[:])
            for t in range(NT):
                tokid = rt.tile([128, 1], I32, tag="tokid")
                nc.gpsimd.iota(tokid[:], pattern=[[1, 1]], base=t * 128,
                               channel_multiplier=1)
                for s in range(2):
                    eqa = (eq1a, eq2a)[s]
                    ga = (g1a, g2a)[s]
                    ef = (e1f, e2f)[s]
                    tmp = rt.tile([128, E], F32, tag="stmp")
                    psel = rt.tile([128, 1], F32, tag="psel")
                    nc.vector.tensor_tensor(tmp[:], pos[:, t, :], eqa[:, t, :],
                                            op=ALU.mult)
                    nc.vector.reduce_sum(psel[:], tmp[:],
                                         axis=mybir.AxisListType.X)
                    of = rt.tile([128, 1], F32, tag="of")
                    nc.vector.tensor_scalar(of[:], ef[:, t:t + 1], float(CAP),
                                            None, op0=ALU.mult)
                    nc.vector.tensor_tensor(of[:], of[:], psel[:], op=ALU.add)
                    oi = rt.tile([128, 1], I32, tag="oi")
                    nc.vector.tensor_copy(oi[:], of[:])
                    sc_in = rt.tile([128, 2], F32, tag="scin")
                    nc.vector.tensor_copy(sc_in[:, 0:1], tokid[:])
                    nc.vector.tensor_copy(sc_in[:, 1:2], ga[:, t:t + 1])
                    nc.gpsimd.indirect_dma_start(
                        out=tbl_ap,
                        out_offset=bass.IndirectOffsetOnAxis(ap=oi[:, :1],
                                                             axis=0),
                        in_=sc_in[:],
                        in_offset=None)

            gidx_f = rt.tile([128, E * CAP // 16], F32, tag="gidxf")
            for r in range(8):
                nc.sync.dma_start(
                    out=gidx_f[r * 16:(r + 1) * 16, :],
                    in_=bass.AP(tbl, 0, [[2, 16], [32, E * CAP // 16]]))
            nc.vector.tensor_copy(gidx[:], gidx_f[:])
            nc.sync.dma_start(
                out=gval[:],
                in_=bass.AP(tbl, 1, [[2, 128], [256, E * NCT]]))

            TCH = 256
            for tc_i in range(TOK // TCH):
                t0 = tc_i * TCH
                psy = [[psY.tile([128, 512], F32, tag="y",
                                 name=f"psy_{tc_i}_{m}_{ch}")
                        for ch in range(2)] for m in range(2)]
                for k in range(KH):
                    wfcs_k = sh.tile([128, KC, 128], BF16, tag="wfcsk")
                    nc.sync.dma_start(
                        out=wfcs_k[:],
                        in_=wfcs[:, k * 128:(k + 1) * 128]
                            .rearrange("(c p) h -> p c h", p=128))
                    ps = psA.tile([128, TCH], F32, tag="acc")
                    for c in range(KC):
                        nc.tensor.matmul(ps[:], lhsT=wfcs_k[:, c, :],
                                         rhs=xt_sb[:, c, t0:t0 + TCH],
                                         start=(c == 0), stop=(c == KC - 1))
                    hsk = sh.tile([128, TCH], BF16, tag="hsk")
                    nc.scalar.activation(hsk[:], ps[:], AF.Gelu_apprx_tanh,
                                         bias=bfcs_sb[:, k:k + 1], scale=1.0)
                    wpjs_k = sh.tile([128, C], BF16, tag="wpjsk")
                    nc.sync.dma_start(out=wpjs_k[:],
                                      in_=wpjs[k * 128:(k + 1) * 128, :])
                    for m in range(2):
                        for ch in range(2):
                            nc.tensor.matmul(
                                psy[m][ch][:],
                                lhsT=hsk[:, m * 128:(m + 1) * 128],
                                rhs=wpjs_k[:, ch * 512:(ch + 1) * 512],
                                start=(k == 0), stop=False)
                for m in range(2):
                    for ch in range(2):
                        nc.tensor.matmul(
                            psy[m][ch][:],
                            lhsT=gateT[:, t0 + m * 128:t0 + (m + 1) * 128],
                            rhs=bias9[:, ch * 512:(ch + 1) * 512],
                            start=False, stop=True)
                yo = sh.tile([128, 2, C], F32, tag="yo")
                for m in range(2):
                    for ch in range(2):
                        nc.vector.tensor_copy(
                            yo[:, m, ch * 512:(ch + 1) * 512], psy[m][ch][:])
                nc.sync.dma_start(
                    out=y[t0:t0 + TCH, :].rearrange("(m p) c -> p m c", p=128),
                    in_=yo[:])

            for e in range(E):
                teT = te.tile([128, KC, CAP], BF16, tag="teT")
                nc.gpsimd.dma_gather(
                    out_ap=teT[:], in_ap=xr,
                    idxs_ap=gidx[:, e * (CAP // 16):(e + 1) * (CAP // 16)],
                    num_idxs=CAP, num_idxs_reg=CAP, elem_size=C,
                    transpose=True)
                heT = he.tile([128, KH, CAP], BF16, tag="heT")
                for hs in range(8):
                    wfc_t = wf.tile([128, KC, 512], BF16, tag="wfct")
                    nc.sync.dma_start(
                        out=wfc_t[:],
                        in_=wfc[e][:, hs * 512:(hs + 1) * 512]
                            .rearrange("(c p) h -> p c h", p=128))
                    for m in range(4):
                        ps = psA.tile([128, CAP], F32, tag="acc")
                        for c in range(KC):
                            nc.tensor.matmul(
                                ps[:],
                                lhsT=wfc_t[:, c, m * 128:(m + 1) * 128],
                                rhs=teT[:, c, :],
                                start=(c == 0), stop=(c == KC - 1))
                        hidx = hs * 4 + m
                        nc.scalar.activation(
                            heT[:, hidx, :], ps[:], AF.Gelu_apprx_tanh,
                            bias=bfc_sb[:, e * KH + hidx:e * KH + hidx + 1],
                            scale=1.0)
                osc = osp.tile([128, NCT, C], F32, tag="osc")
                for ch in range(2):
                    wpjh = wp.tile([128, KH, 512], BF16, tag="wpjh")
                    nc.sync.dma_start(
                        out=wpjh[:],
                        in_=wpj[e][:, ch * 512:(ch + 1) * 512]
                            .rearrange("(k p) c -> p k c", p=128))
                    for m in range(NCT):
                        ps2 = psY.tile([128, 512], F32, tag="y")
                        for k in range(KH):
                            nc.tensor.matmul(
                                ps2[:],
                                lhsT=heT[:, k, m * 128:(m + 1) * 128],
                                rhs=wpjh[:, k, :],
                                start=(k == 0), stop=(k == KH - 1))
                        nc.scalar.mul(
                            osc[:, m, ch * 512:(ch + 1) * 512], ps2[:],
                            mul=gval[:, e * NCT + m:e * NCT + m + 1])
                nc.gpsimd.dma_scatter_add(
                    out_ap=y, in_ap=osc[:],
                    idxs_ap=gidx[:, e * (CAP // 16):(e + 1) * (CAP // 16)],
                    num_idxs=CAP, num_idxs_reg=CAP, elem_size=C)

    nc.compile()
    return nc


def get_nc():
    if "nc" not in _NC_CACHE:
        _NC_CACHE["nc"] = _build_nc()
    return _NC_CACHE["nc"]


def _prep_in_maps(x, Wfc_s, bfc_s, Wproj_s, bproj_s, Wr, Wfc, bfc, Wproj,
                  bproj):
    bf16 = ml_dtypes.bfloat16
    xf = np.ascontiguousarray(np.asarray(x, np.float32).reshape(B * T, C))
    wfc_b = np.ascontiguousarray(np.asarray(Wfc, np.float32)).astype(bf16)
    wpj_b = np.ascontiguousarray(np.asarray(Wproj, np.float32)).astype(bf16)
    wfcs_b = np.ascontiguousarray(np.asarray(Wfc_s, np.float32)).astype(bf16)
    wpjs_b = np.ascontiguousarray(np.asarray(Wproj_s, np.float32)).astype(bf16)
    wr_f = np.ascontiguousarray(np.asarray(Wr, np.float32))
    bfc_f = np.ascontiguousarray(np.asarray(bfc, np.float32))
    bfcs_f = np.ascontiguousarray(np.asarray(bfc_s, np.float32))
    b9 = np.concatenate([np.asarray(bproj, np.float32),
                         np.asarray(bproj_s, np.float32)[None, :]], axis=0)
    b9 = np.ascontiguousarray(b9)

    in_maps = []
    for c in range(N_CORES):
        xs = xf[c * TOK:(c + 1) * TOK]
        xts = np.ascontiguousarray(xs.T)
        in_maps.append({
            "xr": np.ascontiguousarray(xs.astype(bf16)),
            "xtf": xts,
            "xt": np.ascontiguousarray(xts.astype(bf16)),
            "wr": wr_f,
            "wfc": wfc_b,
            "wpj": wpj_b,
            "wfcs": wfcs_b,
            "wpjs": wpjs_b,
            "bfc": bfc_f,
            "bfcs": bfcs_f,
            "b9": b9,
        })
    return in_maps


def kernel(x, Wfc_s, bfc_s, Wproj_s, bproj_s, Wr, Wfc, bfc, Wproj, bproj):
    from concourse.bass_utils import run_bass_kernel_spmd

    nc = get_nc()
    in_maps = _prep_in_maps(x, Wfc_s, bfc_s, Wproj_s, bproj_s, Wr, Wfc, bfc,
                            Wproj, bproj)
    res = run_bass_kernel_spmd(nc, in_maps, core_ids=list(range(N_CORES)))
    out = np.concatenate([res.results[c]["y"] for c in range(N_CORES)], axis=0)
    return out.reshape(B, T, C).astype(np.float32)


# revision 13
# speedup vs baseline: 17185.6413x; 17185.6413x over previous
import numpy as np
import ml_dtypes

B, T = 4, 2048
TOK = 1024
C = 1024
H = 4096
E = 8
CAP = 384
N_CORES = 8
NT = TOK // 128
KC = C // 128
KH = H // 128
NCT = CAP // 128
CAPE = 384

_NC_CACHE = {}


def _build_nc():
    import concourse.bacc as bacc
    import concourse.mybir as mybir
    import concourse.tile as tile
    from concourse import bass
    from concourse.masks import make_identity

    dt = mybir.dt
    AF = mybir.ActivationFunctionType
    ALU = mybir.AluOpType
    F32, BF16, I16, I32, U32 = dt.float32, dt.bfloat16, dt.int16, dt.int32, dt.uint32

    nc = bacc.Bacc("TRN2", target_bir_lowering=False, debug=False,
                   num_devices=N_CORES)

    xr = nc.dram_tensor("xr", [TOK, C], BF16, kind="ExternalInput").ap()
    xtf = nc.dram_tensor("xtf", [C, TOK], F32, kind="ExternalInput").ap()
    xt = nc.dram_tensor("xt", [C, TOK], BF16, kind="ExternalInput").ap()
    wr = nc.dram_tensor("wr", [C, E], F32, kind="ExternalInput").ap()
    wfc = nc.dram_tensor("wfc", [E, C, H], BF16, kind="ExternalInput").ap()
    wpj = nc.dram_tensor("wpj", [E, H, C], BF16, kind="ExternalInput").ap()
    wfcs = nc.dram_tensor("wfcs", [C, H], BF16, kind="ExternalInput").ap()
    wpjs = nc.dram_tensor("wpjs", [H, C], BF16, kind="ExternalInput").ap()
    bfc = nc.dram_tensor("bfc", [E, H], F32, kind="ExternalInput").ap()
    bfcs = nc.dram_tensor("bfcs", [H], F32, kind="ExternalInput").ap()
    b9 = nc.dram_tensor("b9", [E + 1, C], F32, kind="ExternalInput").ap()
    y = nc.dram_tensor("y", [TOK, C], F32, kind="ExternalOutput").ap()
    tbl = nc.dram_tensor("tbl", [E * CAP, 2], F32, kind="Internal")
    tbl_ap = tbl.ap()

    with tile.TileContext(nc) as tc:
        with tc.tile_pool(name="pp", bufs=1) as pp, \
             tc.tile_pool(name="rt", bufs=1) as rt, \
             tc.tile_pool(name="rw", bufs=2) as rw, \
             tc.tile_pool(name="sh", bufs=2) as sh, \
             tc.tile_pool(name="sw", bufs=1) as sw, \
             tc.tile_pool(name="te", bufs=2) as te, \
             tc.tile_pool(name="he", bufs=1) as he, \
             tc.tile_pool(name="wf", bufs=3) as wf, \
             tc.tile_pool(name="wp", bufs=2) as wp, \
             tc.tile_pool(name="os", bufs=1) as osp, \
             tc.tile_pool(name="psA", bufs=3, space="PSUM") as psA, \
             tc.tile_pool(name="psT", bufs=1, space="PSUM") as psT, \
             tc.tile_pool(name="psY", bufs=4, space="PSUM") as psY:

            xt_sb = pp.tile([128, KC, TOK], BF16)
            gidx = pp.tile([128, E * CAP // 16], I16)
            gval = pp.tile([128, E * NCT], F32)
            gateT = pp.tile([E + 1, TOK], F32)
            bias9 = pp.tile([E + 1, C], F32)
            bfc_sb = pp.tile([128, E * KH], F32)
            bfcs_sb = pp.tile([128, KH], F32)
            idn = pp.tile([128, 128], F32)
            ones_m = pp.tile([128, 128], F32)
            sut_m = pp.tile([128, 128], F32)
            iota8 = pp.tile([128, 8], F32)

            nc.sync.dma_start(out=xt_sb[:],
                              in_=xt.rearrange("(k p) t -> p k t", p=128))
            nc.sync.dma_start(out=bias9[:], in_=b9)
            nc.sync.dma_start(out=bfc_sb[:],
                              in_=bfc.rearrange("e (m p) -> p (e m)", p=128))
            nc.sync.dma_start(out=bfcs_sb[:],
                              in_=bfcs.rearrange("(m p) -> p m", p=128))
            make_identity(nc, idn[:])
            nc.gpsimd.memset(ones_m[:], 1.0)
            nc.gpsimd.memset(sut_m[:], 1.0)
            nc.gpsimd.affine_select(out=sut_m[:], in_=sut_m[:],
                                    compare_op=ALU.is_gt, fill=0.0, base=0,
                                    pattern=[[1, 128]], channel_multiplier=-1)
            iota8_i = rt.tile([128, 8], I32, tag="iotai")
            nc.gpsimd.iota(iota8_i[:], pattern=[[1, 8]], base=0,
                           channel_multiplier=0)
            nc.vector.tensor_copy(iota8[:], iota8_i[:])

            wr_sb = rt.tile([128, KC, E], F32, tag="wr")
            nc.sync.dma_start(out=wr_sb[:],
                              in_=wr.rearrange("(k p) e -> p k e", p=128))
            logit = rt.tile([128, NT, E], F32, tag="logit")
            for t in range(NT):
                xtf_t = rw.tile([128, KC, 128], F32, tag="xtf")
                nc.sync.dma_start(
                    out=xtf_t[:],
                    in_=xtf[:, t * 128:(t + 1) * 128]
                        .rearrange("(k p) t -> p k t", p=128))
                ps = psA.tile([128, E], F32, tag="acc")
                for k in range(KC):
                    nc.tensor.matmul(ps[:], lhsT=xtf_t[:, k, :],
                                     rhs=wr_sb[:, k, :],
                                     start=(k == 0), stop=(k == KC - 1))
                nc.vector.tensor_copy(logit[:, t, :], ps[:])

            mask = rt.tile([128, NT, E], F32, tag="mask")
            gden = rt.tile([128, NT, E + 1], F32, tag="gden")
            eq1a = rt.tile([128, NT, E], F32, tag="eq1")
            eq2a = rt.tile([128, NT, E], F32, tag="eq2")
            e1f = rt.tile([128, NT], F32, tag="e1f")
            e2f = rt.tile([128, NT], F32, tag="e2f")
            g1a = rt.tile([128, NT], F32, tag="g1")
            g2a = rt.tile([128, NT], F32, tag="g2")
            pos = rt.tile([128, NT, E], F32, tag="pos")
            for t in range(NT):
                m8 = rt.tile([128, 8], F32, tag="m8")
                i8 = rt.tile([128, 8], U32, tag="i8")
                nc.vector.max_with_indices(m8[:], i8[:], logit[:, t, :])
                nc.vector.tensor_copy(e1f[:, t:t + 1], i8[:, 0:1])
                nc.vector.tensor_copy(e2f[:, t:t + 1], i8[:, 1:2])
                d12 = rt.tile([128, 1], F32, tag="d12")
                nc.vector.tensor_tensor(d12[:], m8[:, 0:1], m8[:, 1:2],
                                        op=ALU.subtract)
                nc.scalar.activation(g1a[:, t:t + 1], d12[:], AF.Sigmoid)
                nc.scalar.activation(g2a[:, t:t + 1], d12[:], AF.Sigmoid,
                                     scale=-1.0)
                nc.vector.tensor_scalar(eq1a[:, t, :], iota8[:],
                                        e1f[:, t:t + 1], None,
                                        op0=ALU.is_equal)
                nc.vector.tensor_scalar(eq2a[:, t, :], iota8[:],
                                        e2f[:, t:t + 1], None,
                                        op0=ALU.is_equal)
                nc.vector.tensor_tensor(mask[:, t, :], eq1a[:, t, :],
                                        eq2a[:, t, :], op=ALU.add)
                tg1 = rt.tile([128, E], F32, tag="tg1")
                tg2 = rt.tile([128, E], F32, tag="tg2")
                nc.vector.tensor_scalar(tg1[:], eq1a[:, t, :],
                                        g1a[:, t:t + 1], None, op0=ALU.mult)
                nc.vector.tensor_scalar(tg2[:], eq2a[:, t, :],
                                        g2a[:, t:t + 1], None, op0=ALU.mult)
                nc.vector.tensor_tensor(gden[:, t, :E], tg1[:], tg2[:],
                                        op=ALU.add)
                nc.vector.memset(gden[:, t, E:E + 1], 1.0)

            for t in range(NT):
                ps = psA.tile([128, E], F32, tag="acc")
                for k in range(t + 1):
                    nc.tensor.matmul(ps[:],
                                     lhsT=(sut_m[:] if k == t else ones_m[:]),
                                     rhs=mask[:, k, :],
                                     start=(k == 0), stop=(k == t))
                nc.vector.tensor_copy(pos[:, t, :], ps[:])

            for t in range(NT):
                trp = psT.tile([E + 1, 128], F32, tag="tr")
                nc.tensor.transpose(trp[:], gden[:, t, :], idn[:])
                nc.vector.tensor_copy(gateT[0:E + 1, t * 128:(t + 1) * 128],
                                      trp[:])

            ztbl = rt.tile([128, E * CAP // 128, 2], F32, tag="ztbl")
            nc.gpsimd.memset(ztbl[:], 0.0)
            nc.sync.dma_start(
                out=tbl_ap.rearrange("(a p) c -> p a c", p=128),
                in_=ztbl[:])
            for t in range(NT):
                tokid = rt.tile([128, 1], I32, tag="tokid")
                nc.gpsimd.iota(tokid[:], pattern=[[1, 1]], base=t * 128,
                               channel_multiplier=1)
                for s in range(2):
                    eqa = (eq1a, eq2a)[s]
                    ga = (g1a, g2a)[s]
                    ef = (e1f, e2f)[s]
                    tmp = rt.tile([128, E], F32, tag="stmp")
                    psel = rt.tile([128, 1], F32, tag="psel")
                    nc.vector.tensor_tensor(tmp[:], pos[:, t, :], eqa[:, t, :],
                                            op=ALU.mult)
                    nc.vector.reduce_sum(psel[:], tmp[:],
                                         axis=mybir.AxisListType.X)
                    of = rt.tile([128, 1], F32, tag="of")
                    nc.vector.tensor_scalar(of[:], ef[:, t:t + 1], float(CAP),
                                            None, op0=ALU.mult)
                    nc.vector.tensor_tensor(of[:], of[:], psel[:], op=ALU.add)
                    oi = rt.tile([128, 1], I32, tag="oi")
                    nc.vector.tensor_copy(oi[:], of[:])
                    sc_in = rt.tile([128, 2], F32, tag="scin")
                    nc.vector.tensor_copy(sc_in[:, 0:1], tokid[:])
                    nc.vector.tensor_copy(sc_in[:, 1:2], ga[:, t:t + 1])
                    nc.gpsimd.indirect_dma_start(
                        out=tbl_ap,
                        out_offset=bass.IndirectOffsetOnAxis(ap=oi[:, :1],
                                                             axis=0),
                        in_=sc_in[:],
                        in_offset=None)

            gidx_f = rt.tile([128, E * CAP // 16], F32, tag="gidxf")
            for r in range(8):
                nc.sync.dma_start(
                    out=gidx_f[r * 16:(r + 1) * 16, :],
                    in_=bass.AP(tbl, 0, [[2, 16], [32, E * CAP // 16]]))
            nc.vector.tensor_copy(gidx[:], gidx_f[:])
            nc.sync.dma_start(
                out=gval[:],
                in_=bass.AP(tbl, 1, [[2, 128], [256, E * NCT]]))

            for kg in range(4):
                wfcs_g = sw.tile([128, KC, 1024], BF16, tag="wfcsg")
                nc.sync.dma_start(
                    out=wfcs_g[:],
                    in_=wfcs[:, kg * 1024:(kg + 1) * 1024]
                        .rearrange("(c p) h -> p c h", p=128))
                wpjs_g = sw.tile([128, 8, C], BF16, tag="wpjsg")
                nc.sync.dma_start(
                    out=wpjs_g[:],
                    in_=wpjs[kg * 1024:(kg + 1) * 1024, :]
                        .rearrange("(k p) c -> p k c", p=128))
                for tch in range(2):
                    t0 = tch * 512
                    hg = sh.tile([128, 8, 512], BF16, tag="hg")
                    for kk in range(8):
                        k = kg * 8 + kk
                        ps = psA.tile([128, 512], F32, tag="acc",
                                      name=f"shfc_{kg}_{tch}_{kk}")
                        for c in range(KC):
                            nc.tensor.matmul(ps[:], lhsT=wfcs_g[:, c,
                                             kk * 128:(kk + 1) * 128],
                                             rhs=xt_sb[:, c, t0:t0 + 512],
                                             start=(c == 0), stop=(c == KC - 1))
                        nc.scalar.activation(hg[:, kk, :], ps[:],
                                             AF.Gelu_apprx_tanh,
                                             bias=bfcs_sb[:, k:k + 1],
                                             scale=1.0)
                    for m in range(4):
                        yo = sh.tile([128, C], F32, tag="yo")
                        for ch in range(2):
                            ps2 = psA.tile([128, 512], F32, tag="acc",
                                           name=f"shpj_{kg}_{tch}_{m}_{ch}")
                            for kk in range(8):
                                nc.tensor.matmul(
                                    ps2[:],
                                    lhsT=hg[:, kk, m * 128:(m + 1) * 128],
                                    rhs=wpjs_g[:, kk, ch * 512:(ch + 1) * 512],
                                    start=(kk == 0),
                                    stop=(kk == 7 and kg != 0))
                            if kg == 0:
                                nc.tensor.matmul(
                                    ps2[:],
                                    lhsT=gateT[:, t0 + m * 128:
                                               t0 + (m + 1) * 128],
                                    rhs=bias9[:, ch * 512:(ch + 1) * 512],
                                    start=False, stop=True)
                            nc.vector.tensor_copy(
                                yo[:, ch * 512:(ch + 1) * 512], ps2[:])
                        rows = y[t0 + m * 128:t0 + (m + 1) * 128, :]
                        if kg == 0:
                            nc.sync.dma_start(out=rows, in_=yo[:])
                        else:
                            nc.gpsimd.dma_start(out=rows, in_=yo[:],
                                                accum_op=ALU.add)

            osc = osp.tile([128, NCT, C], F32, tag="osc")
            for e in range(E):
                teT = te.tile([128, KC, CAP], BF16, tag="teT")
                nc.gpsimd.dma_gather(
                    out_ap=teT[:], in_ap=xr,
                    idxs_ap=gidx[:, e * (CAP // 16):(e + 1) * (CAP // 16)],
                    num_idxs=CAP, num_idxs_reg=CAP, elem_size=C,
                    transpose=True)
                heT = he.tile([128, KH, CAPE], BF16, tag="heT")
                for hs in range(8):
                    wfc_t = wf.tile([128, KC, 512], BF16, tag="wfct")
                    nc.sync.dma_start(
                        out=wfc_t[:],
                        in_=wfc[e][:, hs * 512:(hs + 1) * 512]
                            .rearrange("(c p) h -> p c h", p=128))
                    for m in range(4):
                        ps = psA.tile([128, CAPE], F32, tag="acc",
                                      name=f"fc_{e}_{hs}_{m}")
                        for c in range(KC):
                            nc.tensor.matmul(
                                ps[:],
                                lhsT=wfc_t[:, c, m * 128:(m + 1) * 128],
                                rhs=teT[:, c, 0:CAPE],
                                start=(c == 0), stop=(c == KC - 1))
                        hidx = hs * 4 + m
                        nc.scalar.activation(
                            heT[:, hidx, :], ps[:], AF.Gelu_apprx_tanh,
                            bias=bfc_sb[:, e * KH + hidx:e * KH + hidx + 1],
                            scale=1.0)
                for ch in range(2):
                    ps2s = [psY.tile([128, 512], F32, tag="y",
                                     name=f"pj_{e}_{ch}_{m}")
                            for m in range(NCT)]
                    for kg in range(4):
                        wpjh = wp.tile([128, 8, 512], BF16, tag="wpjh")
                        nc.sync.dma_start(
                            out=wpjh[:],
                            in_=wpj[e][kg * 1024:(kg + 1) * 1024,
                                       ch * 512:(ch + 1) * 512]
                                .rearrange("(k p) c -> p k c", p=128))
                        for m in range(NCT):
                            mw = 128 if m < 2 else CAPE - 256
                            for kk in range(8):
                                nc.tensor.matmul(
                                    ps2s[m][:mw, :],
                                    lhsT=heT[:, kg * 8 + kk,
                                             m * 128:m * 128 + mw],
                                    rhs=wpjh[:, kk, :],
                                    start=(kg == 0 and kk == 0),
                                    stop=(kg == 3 and kk == 7))
                    for m in range(NCT):
                        mw = 128 if m < 2 else CAPE - 256
                        nc.scalar.mul(
                            osc[:mw, m, ch * 512:(ch + 1) * 512],
                            ps2s[m][:mw, :],
                            mul=gval[:mw, e * NCT + m:e * NCT + m + 1])
                nc.gpsimd.dma_scatter_add(
                    out_ap=y, in_ap=osc[:],
                    idxs_ap=gidx[:, e * (CAP // 16):
                                 e * (CAP // 16) + CAPE // 16],
                    num_idxs=CAPE, num_idxs_reg=CAPE, elem_size=C)

    nc.compile()
    return nc


def get_nc():
    if "nc" not in _NC_CACHE:
        _NC_CACHE["nc"] = _build_nc()
    return _NC_CACHE["nc"]


def _prep_in_maps(x, Wfc_s, bfc_s, Wproj_s, bproj_s, Wr, Wfc, bfc, Wproj,
                  bproj):
    bf16 = ml_dtypes.bfloat16
    xf = np.ascontiguousarray(np.asarray(x, np.float32).reshape(B * T, C))
    wfc_b = np.ascontiguousarray(np.asarray(Wfc, np.float32)).astype(bf16)
    wpj_b = np.ascontiguousarray(np.asarray(Wproj, np.float32)).astype(bf16)
    wfcs_b = np.ascontiguousarray(np.asarray(Wfc_s, np.float32)).astype(bf16)
    wpjs_b = np.ascontiguousarray(np.asarray(Wproj_s, np.float32)).astype(bf16)
    wr_f = np.ascontiguousarray(np.asarray(Wr, np.float32))
    bfc_f = np.ascontiguousarray(np.asarray(bfc, np.float32))
    bfcs_f = np.ascontiguousarray(np.asarray(bfc_s, np.float32))
    b9 = np.concatenate([np.asarray(bproj, np.float32),
                         np.asarray(bproj_s, np.float32)[None, :]], axis=0)
    b9 = np.ascontiguousarray(b9)

    in_maps = []
    for c in range(N_CORES):
        xs = xf[c * TOK:(c + 1) * TOK]
        xts = np.ascontiguousarray(xs.T)
        in_maps.append({
            "xr": np.ascontiguousarray(xs.astype(bf16)),
            "xtf": xts,
            "xt": np.ascontiguousarray(xts.astype(bf16)),
            "wr": wr_f,
            "wfc": wfc_b,
            "wpj": wpj_b,
            "wfcs": wfcs_b,
            "wpjs": wpjs_b,
            "bfc": bfc_f,
            "bfcs": bfcs_f,
            "b9": b9,
        })
    return in_maps


def _check_capacity(x, Wr):
    xf = np.asarray(x, np.float32).reshape(B * T, C)
    logits = xf @ np.asarray(Wr, np.float32)
    part = np.argpartition(-logits, 2, axis=-1)[:, :2]
    for c in range(N_CORES):
        sl = part[c * TOK:(c + 1) * TOK]
        counts = np.bincount(sl.ravel(), minlength=E)
        if counts.max() > CAPE:
            raise RuntimeError(
                f"core {c}: expert token count {counts.max()} exceeds static "
                f"capacity {CAPE}; rebuild kernel with a larger CAP/CAPE")


def kernel(x, Wfc_s, bfc_s, Wproj_s, bproj_s, Wr, Wfc, bfc, Wproj, bproj):
    from concourse.bass_utils import run_bass_kernel_spmd

    _check_capacity(x, Wr)
    nc = get_nc()
    in_maps = _prep_in_maps(x, Wfc_s, bfc_s, Wproj_s, bproj_s, Wr, Wfc, bfc,
                            Wproj, bproj)
    res = run_bass_kernel_spmd(nc, in_maps, core_ids=list(range(N_CORES)))
    out = np.concatenate([res.results[c]["y"] for c in range(N_CORES)], axis=0)
    return out.reshape(B, T, C).astype(np.float32)
